# revision 1
# baseline (speedup 1.0000x reference)
"""Trainium2 Bass kernel: batched bilinear form  out[n] = elg[n] @ W @ eth[n].

Problem: elg, eth [32768, 1024] fp32, W [1024, 1024] fp32.
Sharding: data-parallel over the batch (N) axis across 8 NeuronCores;
W is replicated.  Per core (4096 rows):

    T      = elg @ W                   (TensorE, fp16 in, fp32 PSUM accum)
    out[n] = sum_e T[n,e] * eth[n,e]   (VectorE fused multiply-reduce, fp32)

elg and W are cast to fp16 on the host (values are ~N(0,1); input
quantization error is ~3e-4 relative per element and ~3e-4 of the output
absmax after accumulation — PSUM accumulation itself is fp32).  eth stays
fp32 and the reduction is fp32.

Layout: the matmul contracts over the partition axis, so elg tiles are
loaded pre-transposed [d, n] straight from HBM via the DMA xbar transpose
(2-byte dtype path) — no PE/DVE transpose work at all.  W lives in SBUF
fp16 [128, 8, 1024] for the whole kernel.  Each 128-row output tile takes
16 matmuls (8 k-tiles x 2 psum half-banks of 512 fp32) followed by one
fused affine_mul_reduce against eth.
"""

import numpy as np

N_TOTAL = 32768
D = 1024
N_CORES = 8
N_CORE = N_TOTAL // N_CORES          # 4096 rows per core
P = 128                              # SBUF/PSUM partitions
K_TILES = D // P                     # 8 contraction tiles
CHUNK_ROWS = 1024                    # rows per DMA chunk
TILES_PER_CHUNK = CHUNK_ROWS // P    # 8
E_HALF = 512                         # fp32 free elems per PSUM bank

_CACHE = {}


def _build_program(n_core_rows, repeats=1, chunk_rows=None, lg_bufs=2, et_bufs=2,
                   ps_bufs=3, eth_per_tile=True, head_chunks=()):
    """Build the per-core Bass program.

    chunk_rows: rows per steady-state DMA chunk.
    head_chunks: optional smaller leading chunk sizes (e.g. (256, 768)) so the
        first matmul group's elgT lands sooner; remaining rows use chunk_rows.
    eth_per_tile: load eth as 32 plain [128, D] DMAs instead of chunked
        rearranged DMAs.
    """
    import concourse.tile as tile
    from concourse import bacc, mybir

    f16 = mybir.dt.float16
    f32 = mybir.dt.float32

    if chunk_rows is None:
        chunk_rows = CHUNK_ROWS
    sched = list(head_chunks)
    left = n_core_rows - sum(sched)
    assert left >= 0 and left % chunk_rows == 0
    sched += [chunk_rows] * (left // chunk_rows)
    assert all(c % P == 0 for c in sched)
    n_tiles = n_core_rows // P

    nc = bacc.Bacc("TRN2", target_bir_lowering=False, debug=False)
    elg16 = nc.dram_tensor("elg16", [n_core_rows, D], f16, kind="ExternalInput").ap()
    eth = nc.dram_tensor("eth", [n_core_rows, D], f32, kind="ExternalInput").ap()
    w16 = nc.dram_tensor("w16", [D, D], f16, kind="ExternalInput").ap()
    out = nc.dram_tensor("out", [P, n_tiles * repeats], f32, kind="ExternalOutput").ap()

    max_chunk = max(sched)
    with tile.TileContext(nc) as tc:
        with tc.tile_pool(name="w_pool", bufs=1) as w_pool, \
             tc.tile_pool(name="lg_pool", bufs=lg_bufs) as lg_pool, \
             tc.tile_pool(name="et_pool", bufs=et_bufs) as et_pool, \
             tc.tile_pool(name="pr_pool", bufs=2) as pr_pool, \
             tc.tile_pool(name="acc_pool", bufs=1) as acc_pool, \
             tc.tile_pool(name="ps_pool", bufs=ps_bufs, space="PSUM") as ps_pool:

            w_sb = w_pool.tile([P, K_TILES, D], f16, name="w_sb")
            for k in range(K_TILES):
                nc.sync.dma_start(
                    out=w_sb[:, k, :],
                    in_=w16[k * P:(k + 1) * P, :],
                )

            out_sb = acc_pool.tile([P, n_tiles * repeats], f32, name="out_sb")

            for _rep in range(repeats):
                r0 = 0
                t_idx = _rep * n_tiles
                for chunk in sched:
                    tiles_here = chunk // P
                    elgT = lg_pool.tile([P, K_TILES, max_chunk], f16, name="elgT")
                    for k in range(K_TILES):
                        nc.sync.dma_start(
                            out=elgT[:, k, :chunk],
                            in_=elg16[r0:r0 + chunk, k * P:(k + 1) * P],
                            transpose=True,
                        )
                    if eth_per_tile:
                        eth_sb = et_pool.tile(
                            [P, max_chunk // P, D], f32, name="eth_sb")
                        for s in range(tiles_here):
                            nc.sync.dma_start(
                                out=eth_sb[:, s, :],
                                in_=eth[r0 + s * P:r0 + (s + 1) * P, :],
                            )
                    else:
                        eth_sb = et_pool.tile(
                            [P, max_chunk // P, D], f32, name="eth_sb")
                        nc.sync.dma_start(
                            out=eth_sb[:, :tiles_here, :],
                            in_=eth[r0:r0 + chunk, :].rearrange(
                                "(s p) e -> p s e", p=P),
                        )

                    for s in range(tiles_here):
                        t_ps = ps_pool.tile([P, D], f32, name="t_ps")
                        for k in range(K_TILES):
                            for eh in range(2):
                                nc.tensor.matmul(
                                    t_ps[:, eh * E_HALF:(eh + 1) * E_HALF],
                                    elgT[:, k, s * P:(s + 1) * P],
                                    w_sb[:, k, eh * E_HALF:(eh + 1) * E_HALF],
                                    start=(k == 0),
                                    stop=(k == K_TILES - 1),
                                )
                        prod = pr_pool.tile([P, D], f32, name="prod")
                        nc.vector.affine_mul_reduce(
                            out=prod[:],
                            accum_out=out_sb[:, t_idx:t_idx + 1],
                            in0=t_ps[:],
                            in1=eth_sb[:, s, :],
                            scale=1.0,
                            bias=0.0,
                        )
                        t_idx += 1
                    r0 += chunk

            nc.sync.dma_start(out=out, in_=out_sb[:])

    nc.compile()
    return nc


def _make_runner(nc, n_cores):
    """Mirror bass2jax.run_bass_via_pjrt's multi-core branch, but return a
    cached jitted callable so repeat calls skip retracing.
    """
    import jax
    import concourse.mybir as mybir
    from concourse import bass2jax
    from jax.experimental.shard_map import shard_map
    from jax.sharding import Mesh, PartitionSpec

    bass2jax.install_neuronx_cc_hook()
    assert nc.dbg_addr is None
    partition_name = nc.partition_id_tensor.name if nc.partition_id_tensor else None

    in_names, out_names, out_avals = [], [], []
    for alloc in nc.m.functions[0].allocations:
        if not isinstance(alloc, mybir.MemoryLocationSet):
            continue
        name = alloc.memorylocations[0].name
        if alloc.kind == "ExternalInput":
            if name != partition_name:
                in_names.append(name)
        elif alloc.kind == "ExternalOutput":
            shape = tuple(alloc.tensor_shape)
            dtype = mybir.dt.np(alloc.dtype)
            out_names.append(name)
            out_avals.append(jax.core.ShapedArray(shape, dtype))
    n_params = len(in_names)
    n_outs = len(out_avals)
    all_in_names = in_names + out_names
    if partition_name is not None:
        all_in_names = all_in_names + [partition_name]

    def _body(*args):
        operands = list(args)
        if partition_name is not None:
            operands.append(bass2jax.partition_id_tensor())
        outs = bass2jax._bass_exec_p.bind(
            *operands,
            out_avals=tuple(out_avals),
            in_names=tuple(all_in_names),
            out_names=tuple(out_names),
            lowering_input_output_aliases=(),
            sim_require_finite=True,
            sim_require_nnan=True,
            nc=nc,
        )
        return tuple(outs)

    devices = jax.devices()[:n_cores]
    assert len(devices) == n_cores
    mesh = Mesh(np.asarray(devices), ("core",))
    spec = PartitionSpec("core")
    sharded = jax.jit(
        shard_map(
            _body,
            mesh=mesh,
            in_specs=(spec,) * (n_params + n_outs),
            out_specs=(spec,) * n_outs,
            check_rep=False,
        ),
        donate_argnums=tuple(range(n_params, n_params + n_outs)),
        keep_unused=True,
    )
    zero_out_shapes = [
        ((n_cores * av.shape[0],) + tuple(av.shape[1:]), av.dtype) for av in out_avals
    ]
    return sharded, in_names, out_names, zero_out_shapes, mesh, spec


def _get_runner():
    r = _CACHE.get("runner")
    if r is None:
        nc = _build_program(N_CORE)
        r = _CACHE["runner"] = _make_runner(nc, N_CORES)
    return r


def _global_inputs(elg, eth, weight):
    """Host-side marshalling: cast + per-core-tile the global arrays."""
    elg16 = elg.astype(np.float16)
    w16 = np.broadcast_to(weight.astype(np.float16), (N_CORES, D, D)).reshape(
        N_CORES * D, D
    )
    return {"elg16": elg16, "eth": eth, "w16": w16}


def _call_runner(global_ins):
    sharded, in_names, out_names, zero_out_shapes, _, _ = _get_runner()
    zeros = [np.zeros(shape, dt) for shape, dt in zero_out_shapes]
    out_arrs = sharded(*[global_ins[n] for n in in_names], *zeros)
    out_g = np.asarray(out_arrs[out_names.index("out")])  # [8*128, 32]
    return np.concatenate(
        [out_g[c * P:(c + 1) * P].T.reshape(-1) for c in range(N_CORES)]
    ).astype(np.float32)


def kernel(elg, eth, weight):
    elg = np.asarray(elg, dtype=np.float32)
    eth = np.asarray(eth, dtype=np.float32)
    weight = np.asarray(weight, dtype=np.float32)
    return _call_runner(_global_inputs(elg, eth, weight))



# revision 2
# speedup vs baseline: 1.0076x; 1.0076x over previous
"""Trainium2 Bass kernel: batched bilinear form  out[n] = elg[n] @ W @ eth[n].

Problem: elg, eth [32768, 1024] fp32, W [1024, 1024] fp32.
Data-parallel over the batch (N) axis across 8 NeuronCores (4096 rows per
core); W replicated.

Per-core algorithm ([e, n] "W-stationary" orientation):

    T'[e, n]  = sum_k W[k, e] * elgT[k, n]            (TensorE, fp16)
    acc[p, n] = sum_eb T'[eb*128+p, n] * ethT[eb*128+p, n]   (VectorE, fp16)
    out[n]    = sum_p acc[p, n]                        (ones-matmul + row copy)

Layout / DMA strategy (this environment is DMA-bandwidth-bound at roughly
75-110 GB/s effective, with a ~2 us fixed cost per dma_start):
  * elg and eth are cast to fp16 and pre-transposed/pre-packed on the host
    into per-(chunk) partition-major contiguous blocks, so each chunk of
    each tensor is ONE large (1-4 MB) dma_start whose per-partition bytes
    are a single contiguous run.  6 input dma_starts per pass instead of
    dozens of strided ones.
  * elg chunks issue on the SP HWDGE ring (nc.sync), eth chunks on the
    ACT HWDGE ring (nc.scalar).
  * chunks (512, 1536, 2048) keep the first matmul group's data arriving
    early while the later, bigger chunks amortize overhead.

Compute structure:
  * stationary operand = 128x128 W tile (reused across the chunk's
    n-blocks), moving operand = elgT columns; accumulation over the 8
    k-tiles in PSUM fp32.  Two [128, 2048] psum tiles ping-pong so the
    VectorE drain of one eb-block overlaps the matmuls of the next.
  * VectorE multiplies each psum block by ethT and accumulates in fp16.
  * The final 128-partition reduction is 8 N=512 matmuls against a ones
    [128,128] stationary + a row copy -- ~2 us, mostly hidden.

Numerics: fp16 inputs / fp32 psum accumulation; measured full-problem
relative error ~7e-4 (gate 2e-2).
"""

import numpy as np

N_TOTAL = 32768
D = 1024
N_CORES = 8
N_CORE = N_TOTAL // N_CORES          # 4096 rows per core
P = 128
K_TILES = D // P                     # 8
E_TILES = D // P                     # 8
NB = 512                             # matmul moving free-dim (1 psum bank)
CHUNKS = (512, 1536, 2048)

_CACHE = {}


def _build_program(n_core_rows=N_CORE, repeats=1, chunks=None,
                   ps_bufs=2, lg_bufs=2, et_bufs=2):
    import concourse.tile as tile
    from concourse import bacc, mybir

    f16 = mybir.dt.float16
    f32 = mybir.dt.float32

    if chunks is None:
        chunks = CHUNKS
    assert sum(chunks) == n_core_rows
    assert all(c % NB == 0 for c in chunks)
    cmax = max(chunks)
    L = D * n_core_rows

    nc = bacc.Bacc("TRN2", target_bir_lowering=False, debug=False)
    elgT = nc.dram_tensor("elgT16", [L], f16, kind="ExternalInput").ap()
    ethT = nc.dram_tensor("ethT16", [L], f16, kind="ExternalInput").ap()
    w16 = nc.dram_tensor("w16", [D * D], f16, kind="ExternalInput").ap()
    out = nc.dram_tensor("out", [1, n_core_rows * repeats], f32,
                         kind="ExternalOutput").ap()

    with tile.TileContext(nc) as tc:
        with tc.tile_pool(name="w_pool", bufs=1) as w_pool, \
             tc.tile_pool(name="one_pool", bufs=1) as one_pool, \
             tc.tile_pool(name="lg_pool", bufs=lg_bufs) as lg_pool, \
             tc.tile_pool(name="et_pool", bufs=et_bufs) as et_pool, \
             tc.tile_pool(name="pr_pool", bufs=3) as pr_pool, \
             tc.tile_pool(name="acc_pool", bufs=2) as acc_pool, \
             tc.tile_pool(name="os_pool", bufs=2) as os_pool, \
             tc.tile_pool(name="ps_pool", bufs=ps_bufs, space="PSUM") as ps_pool:

            w_sb = w_pool.tile([P, K_TILES, D], f16, name="w_sb")
            nc.sync.dma_start(
                out=w_sb[:, :, :],
                in_=w16.rearrange("(p k e) -> p k e", p=P, k=K_TILES))
            ones_sb = one_pool.tile([P, P], f16, name="ones_sb")
            nc.vector.memset(ones_sb[:], 1.0)

            for _rep in range(repeats):
                acc = acc_pool.tile([P, n_core_rows], f16, name="acc")
                out_sb = os_pool.tile([1, n_core_rows], f32, name="out_sb")
                base = 0
                for C in chunks:
                    lg = lg_pool.tile([P, K_TILES, cmax], f16, name="lg")
                    nc.sync.dma_start(
                        out=lg[:, :, :C],
                        in_=elgT[base * D:(base + C) * D].rearrange(
                            "(p k c) -> p k c", p=P, k=K_TILES))
                    et = et_pool.tile([P, E_TILES, cmax], f16, name="et")
                    nc.scalar.dma_start(
                        out=et[:, :, :C],
                        in_=ethT[base * D:(base + C) * D].rearrange(
                            "(p k c) -> p k c", p=P, k=E_TILES))
                    for eb in range(E_TILES):
                        ps = ps_pool.tile([P, cmax], f32, name="ps")
                        for kb in range(K_TILES):
                            for nb in range(C // NB):
                                nc.tensor.matmul(
                                    ps[:, nb * NB:(nb + 1) * NB],
                                    w_sb[:, kb, eb * P:(eb + 1) * P],
                                    lg[:, kb, nb * NB:(nb + 1) * NB],
                                    start=(kb == 0),
                                    stop=(kb == K_TILES - 1),
                                )
                        if eb == 0:
                            nc.vector.tensor_mul(
                                acc[:, base:base + C], ps[:, :C],
                                et[:, 0, :C])
                        else:
                            prod = pr_pool.tile([P, cmax], f16, name="prod")
                            nc.vector.tensor_mul(
                                prod[:, :C], ps[:, :C], et[:, eb, :C])
                            nc.vector.tensor_add(
                                acc[:, base:base + C],
                                acc[:, base:base + C], prod[:, :C])
                    base += C

                # final 128-partition reduce: ones-matmul + row copy
                RW = min(2048, cmax)
                for h in range(n_core_rows // RW):
                    out_f = ps_pool.tile([P, cmax], f32, name="ps")
                    for nb in range(RW // NB):
                        nc.tensor.matmul(
                            out_f[:, nb * NB:(nb + 1) * NB],
                            ones_sb[:, :],
                            acc[:, h * RW + nb * NB:h * RW + (nb + 1) * NB],
                            start=True, stop=True,
                        )
                    nc.vector.tensor_copy(
                        out_sb[0:1, h * RW:(h + 1) * RW], out_f[0:1, :RW])
                nc.scalar.dma_start(
                    out=out[0:1, _rep * n_core_rows:(_rep + 1) * n_core_rows],
                    in_=out_sb[0:1, :])

    nc.compile()
    return nc


def _make_runner(nc, n_cores):
    """Cached jitted shard_map over the bass_exec primitive (axon/PJRT)."""
    import jax
    import concourse.mybir as mybir
    from concourse import bass2jax
    from jax.experimental.shard_map import shard_map
    from jax.sharding import Mesh, PartitionSpec

    bass2jax.install_neuronx_cc_hook()
    assert nc.dbg_addr is None
    partition_name = nc.partition_id_tensor.name if nc.partition_id_tensor else None

    in_names, out_names, out_avals = [], [], []
    for alloc in nc.m.functions[0].allocations:
        if not isinstance(alloc, mybir.MemoryLocationSet):
            continue
        name = alloc.memorylocations[0].name
        if alloc.kind == "ExternalInput":
            if name != partition_name:
                in_names.append(name)
        elif alloc.kind == "ExternalOutput":
            shape = tuple(alloc.tensor_shape)
            dtype = mybir.dt.np(alloc.dtype)
            out_names.append(name)
            out_avals.append(jax.core.ShapedArray(shape, dtype))
    n_params = len(in_names)
    n_outs = len(out_avals)
    all_in_names = in_names + out_names
    if partition_name is not None:
        all_in_names = all_in_names + [partition_name]

    def _body(*args):
        operands = list(args)
        if partition_name is not None:
            operands.append(bass2jax.partition_id_tensor())
        outs = bass2jax._bass_exec_p.bind(
            *operands,
            out_avals=tuple(out_avals),
            in_names=tuple(all_in_names),
            out_names=tuple(out_names),
            lowering_input_output_aliases=(),
            sim_require_finite=True,
            sim_require_nnan=True,
            nc=nc,
        )
        return tuple(outs)

    devices = jax.devices()[:n_cores]
    assert len(devices) == n_cores
    mesh = Mesh(np.asarray(devices), ("core",))
    spec = PartitionSpec("core")
    sharded = jax.jit(
        shard_map(
            _body,
            mesh=mesh,
            in_specs=(spec,) * (n_params + n_outs),
            out_specs=(spec,) * n_outs,
            check_rep=False,
        ),
        donate_argnums=tuple(range(n_params, n_params + n_outs)),
        keep_unused=True,
    )
    zero_out_shapes = [
        ((n_cores * av.shape[0],) + tuple(av.shape[1:]), av.dtype) for av in out_avals
    ]
    return sharded, in_names, out_names, zero_out_shapes, mesh, spec


def _get_runner():
    r = _CACHE.get("runner")
    if r is None:
        nc = _build_program(N_CORE)
        r = _CACHE["runner"] = _make_runner(nc, N_CORES)
    return r


def _pack_chunks(xT):
    """xT: [N_CORES, D, N_CORE] -> flat [N_CORES*D*N_CORE] with per-(core,
    chunk) partition-major contiguous blocks [128p, 8k, C]."""
    parts = []
    base = 0
    for C in CHUNKS:
        blk = xT[:, :, base:base + C].reshape(N_CORES, K_TILES, P, C)
        parts.append(np.ascontiguousarray(blk.transpose(0, 2, 1, 3)
                                          ).reshape(N_CORES, -1))
        base += C
    return np.ascontiguousarray(np.concatenate(parts, axis=1)).reshape(-1)


def _global_inputs(elg, eth, weight):
    elgT = elg.astype(np.float16).reshape(N_CORES, N_CORE, D).transpose(0, 2, 1)
    ethT = eth.astype(np.float16).reshape(N_CORES, N_CORE, D).transpose(0, 2, 1)
    w_pm = np.ascontiguousarray(
        weight.astype(np.float16).reshape(K_TILES, P, D).transpose(1, 0, 2)
    ).reshape(-1)  # [128p, 8k, 1024e] flat
    w16 = np.broadcast_to(w_pm, (N_CORES, D * D)).reshape(-1)
    return {"elgT16": _pack_chunks(elgT), "ethT16": _pack_chunks(ethT),
            "w16": w16}


def _call_runner(global_ins):
    sharded, in_names, out_names, zero_out_shapes, _, _ = _get_runner()
    zeros = [np.zeros(shape, dt) for shape, dt in zero_out_shapes]
    out_arrs = sharded(*[global_ins[n] for n in in_names], *zeros)
    out_g = np.asarray(out_arrs[out_names.index("out")])  # [8, 4096]
    return out_g.reshape(-1).astype(np.float32)


def kernel(elg, eth, weight):
    elg = np.asarray(elg, dtype=np.float32)
    eth = np.asarray(eth, dtype=np.float32)
    weight = np.asarray(weight, dtype=np.float32)
    return _call_runner(_global_inputs(elg, eth, weight))
